# revision 35
# baseline (speedup 1.0000x reference)
"""Trainium2 Bass kernel for MixtureOfSoftmaxes.

Module: RMSNorm -> gate MLP (silu, softmax over K experts) -> big GEMM
x @ expert_w (H=1024 -> K*V=128000), softmax over V per expert, mix with
gate weights, log.

Sharding: tensor-parallel over vocab. Core c owns, for all K=4 experts,
the vocab window [c*4000, (c+1)*4000). The only cross-core quantity is
the per-(token, expert) softmax denominator Z.

v3 design (single fused NEFF):
- RMSNorm folded into host prep: x-hat = x/rms(x) is computed in numpy,
  shipped pre-transposed (h on partitions) in fp8 (1 MB). No on-device
  norm, no PE transposes, no psum transpose bank.
- Block-0 GEMM starts as soon as w group 0 lands (~8us) and is paced by
  the weight-shard DMA (16 MB fp8, ~47us wire). The gate MLP's matmuls
  are emitted between block-0 groups so they fill the w-arrival stalls.
- GEMM per (token block, w group): 2 psum sub-tiles of [128,1024] f32
  (1024+1024 or 1024+928 cols), 8 DoubleRow fp8 matmuls each, then one
  Exp activation per sub-tile (psum -> fp8 P in SBUF) with accum_out
  giving quarter row-sums.
- Cross-core reduction via AllGather (4.6us floor @8 cores vs 9.7us for
  AllReduce, and single-phase so the CC queue drains ~2x faster) of the
  [128,K] partial sums; the 8 gathered slices are pair-summed locally on
  vector (3 adds). The old AllReduce measured 35-67us bi->z4 latency and
  its CC-queue serialization both stalled the GEMM pipeline mid-kernel
  and dominated the tail.
- P is kept in fp8 (16 KB/partition/block, 3 bufs); mix runs two blocks
  behind the GEMM; the two exposed tail mixes split their products
  between vector and scalar.
- Output is written bf16 (halves out-DMA bytes; ~0.2% extra rel err vs
  the 2e-2 gate) and upcast to f32 on host.
- DMA triggers that can wait (z return, outputs) live on the sync queue;
  gate-weight loads trigger from the gpsimd queue so the sync queue's
  head is free for x/w.
"""

import sys

sys.path.insert(0, "/opt/trn_rl_repo")

import numpy as np
import ml_dtypes

import concourse.bass as bass
import concourse.bacc as bacc
import concourse.mybir as mybir
import concourse.tile as tile
import concourse.hw_specs as hw_specs
import concourse.bass_interp as bass_interp
from concourse.bass_utils import run_bass_kernel_spmd
from concourse.masks import make_identity

# Steer the act-table-set selector to the combined exp+ln set: with the
# separate "exp_and_others"/"natural_log" sets visible, every Exp<->Ln
# transition in the steady loop re-loads the table (~2.7us of
# ACT_TABLE_LOAD + drain, twice per block, and the contiguous scalar
# pause stalls the GEMM through psum backpressure). Dropping those sets
# makes both functions resolve to "natural_log_exp_and_others", so the
# table is loaded once and never switched mid-loop.
import functools


@functools.cache
def _gat_combined(module_arch):
    # keep every set (ids are positional) but claim exp/ln only in the
    # combined set so the selector can't pick the single-function ones
    exp_aft = mybir.ActivationFunctionType.Exp
    ln_aft = mybir.ActivationFunctionType.Ln
    t = {}
    for name, fns in _gat_orig(module_arch).items():
        if name != "natural_log_exp_and_others":
            fns = fns - {exp_aft, ln_aft}
        t[name] = fns
    return t


_gat_orig = hw_specs.get_activation_tables
if hw_specs.get_activation_tables is not _gat_combined:
    hw_specs.get_activation_tables = _gat_combined
    bacc.get_activation_tables = _gat_combined
    bass_interp.get_activation_tables = _gat_combined

AFT = mybir.ActivationFunctionType
F32 = mybir.dt.float32
BF16 = mybir.dt.bfloat16
FP8 = mybir.dt.float8e4
FP8NP = ml_dtypes.float8_e4m3
WSCALE = 16.0

B, S, H, K, V = 2, 512, 1024, 4, 32000
T = B * S              # 1024 tokens
NC = 8                 # cores
VSH = V // NC          # 4000 vocab cols per core per expert
C = K * VSH            # 16000 GEMM cols per core (no padding)
D = H // 2             # 512 gate hidden
EPS_RMS = 1e-5
EPS_LOG = 1e-10
TB = T // 128          # 8 token blocks
HB = H // 128          # 8 contraction blocks
# w-layout column groups: per expert [0:2048] and [2048:4000]
GRPS = []
for k in range(K):
    GRPS.append((k * VSH, 2048))
    GRPS.append((k * VSH + 2048, VSH - 2048))
NG = len(GRPS)         # 8 groups
# mix sub-chunks and Ln/out chunks per vocab window
OCH = [(0, 1000), (1000, 1000), (2000, 1000), (3000, 1000)]
OCW = 1024
LNCH = [(0, 1000), (1000, 1000), (2000, 1000), (3000, 1000)]


def build_fused():
    nc = bacc.Bacc("TRN2", target_bir_lowering=False, debug=False, num_devices=NC)
    # x ships pre-normed and pre-transposed (h on partitions) in fp8:
    # partition p holds rows {hb*128+p} of x-hat^T, hb-major. One 1 MB DMA.
    x_d = nc.dram_tensor("x", [128, HB * T], FP8, kind="ExternalInput")
    # w is host-packed per-partition-contiguous: for each column group g,
    # a [128, HB*cw] block where partition p holds rows {hb*128+p} of the
    # group's columns, hb-major.
    w_d = nc.dram_tensor("w", [128, HB * C], FP8, kind="ExternalInput")
    wd_d = nc.dram_tensor("wd", [H, D], FP8, kind="ExternalInput")
    wu_d = nc.dram_tensor("wu", [D, K], BF16, kind="ExternalInput")
    bd_d = nc.dram_tensor("bd", [D, 1], F32, kind="ExternalInput")
    bdn_d = nc.dram_tensor("bdn", [D, 1], F32, kind="ExternalInput")
    bu_d = nc.dram_tensor("bu", [K, 1], F32, kind="ExternalInput")
    o_d = nc.dram_tensor("o", [TB, 128, VSH], BF16, kind="ExternalOutput")

    wd_ap = wd_d.rearrange("(hb p) d -> p hb d", p=128)
    wu_ap = wu_d.rearrange("(db p) k -> p db k", p=128)
    bd_ap = bd_d.rearrange("(db p) o -> p db o", p=128)

    with tile.TileContext(nc) as tc:
        with tc.tile_pool(name="persist", bufs=1) as pers:
            # create ALL persistent tiles up front, BEFORE any scoped pool
            # opens (later pers tiles would inherit false WAR deps from
            # scoped pools occupying the same addresses).
            w_sb = []
            for g, (c0, cw) in enumerate(GRPS):
                w_sb.append(pers.tile([128, HB, cw], FP8, name=f"wg{g}"))
            xT8 = pers.tile([128, HB, T], FP8)     # 8 KB/partition
            ident32 = pers.tile([4, 4], F32)
            make_identity(nc, ident32[:])
            eps_log = pers.tile([128, 1], F32)
            nc.gpsimd.memset(eps_log[:], EPS_LOG)
            gw = pers.tile([128, TB, K], F32)
            # group sums laid out [t, half, expert] so the per-expert
            # pair-add is a plain elementwise add of the two halves
            schunk = pers.tile([128, TB, 2, K], F32)

            with tc.tile_pool(name="pfull", bufs=3) as ppool, \
                 tc.tile_pool(name="agq", bufs=2) as agq, \
                 tc.tile_pool(name="ccdr", bufs=2, space="DRAM") as ccdr, \
                 tc.tile_pool(name="mm_psum", bufs=2, space="PSUM") as mmps:

                # gate-only SBUF lives in its own scope that closes after
                # the gate finishes; the mix pool opens after and reuses
                # its addresses (they never coexist), which is what funds
                # the third P buffer for the distance-2 pipeline.
                gsb_ctx = tc.tile_pool(name="gate_sb", bufs=1)
                gsb = gsb_ctx.__enter__()

                # warm up the collective path: the first collective pays
                # ~45us of NEFF staging + entry-barrier latency; a dummy
                # AllGather at t=0 absorbs that under the weight DMA so
                # the per-block gathers run at the ~12us steady latency.
                wbi = ccdr.tile([128, K], F32, tag="warm", name="wbi", bufs=1)
                wbo = ccdr.tile([NC, 128, K], F32, tag="warmo", name="wbo",
                                bufs=1)
                nc.gpsimd.collective_compute(
                    "AllGather", mybir.AluOpType.bypass,
                    replica_groups=[list(range(NC))],
                    ins=[wbi[:]], outs=[wbo[:]],
                )

                # ---- input DMAs ----
                # sync queue: x first (gates block-0 GEMM), then the w
                # groups. Cap descriptors at 4 KB so they round-robin
                # across all 16 DMA engines.
                nc.sync.dma_start(xT8[:], x_d[:].rearrange("p (h t) -> p h t", h=HB))
                off = 0
                for g, (c0, cw) in enumerate(GRPS):
                    half = HB // 2 * cw
                    for j in range(2):
                        nc.sync.dma_start(
                            w_sb[g][:, j * HB // 2 : (j + 1) * HB // 2, :],
                            w_d[:, off + j * half : off + (j + 1) * half].rearrange(
                                "p (h c) -> p h c", h=HB // 2),
                            max_dma_last_dim=4096)
                    off += HB * cw
                # gate weights trigger from the gpsimd queue (idle early)
                wd_sb = gsb.tile([128, HB, D], FP8)   # 4 KB/partition
                nc.gpsimd.dma_start(wd_sb[:], wd_ap)
                wu_sb = gsb.tile([128, D // 128, K], BF16)
                nc.gpsimd.dma_start(wu_sb[:], wu_ap)
                bd_sb = gsb.tile([128, D // 128, 1], F32)
                nc.gpsimd.dma_start(bd_sb[:], bd_ap)
                bdn_sb = gsb.tile([128, D // 128, 1], F32)
                nc.gpsimd.dma_start(bdn_sb[:],
                                    bdn_d.rearrange("(db p) o -> p db o", p=128))
                bu_sb = gsb.tile([K, 1], F32)
                nc.gpsimd.dma_start(bu_sb[:], bu_d[:])

                pts = {}

                def emit_gemm_group(t, g):
                    pt = pts[t]
                    c0, cw = GRPS[g]
                    PT = mmps.tile([128, 2048], F32, tag="mm",
                                   name=f"mm_{t}_{g}")
                    for hs in range(HB // 2):
                        for ch0 in range(0, cw, 512):
                            chw = min(512, cw - ch0)
                            nc.tensor.matmul(
                                PT[:, ch0 : ch0 + chw],
                                lhsT=xT8[:, 2 * hs : 2 * hs + 2,
                                         t * 128 : (t + 1) * 128],
                                rhs=w_sb[g][:, 2 * hs : 2 * hs + 2,
                                            ch0 : ch0 + chw],
                                start=(hs == 0), stop=(hs == HB // 2 - 1),
                                perf_mode=mybir.MatmulPerfMode.DoubleRow,
                            )
                    nc.scalar.activation(
                        pt[:, c0 : c0 + cw], PT[:, :cw],
                        AFT.Exp, bias=0.0, scale=1.0 / WSCALE,
                        accum_out=schunk[:, t, g % 2, g // 2 : g // 2 + 1])

                def emit_gemm(t, groups=range(NG)):
                    if t not in pts:
                        pts[t] = ppool.tile([128, C], FP8, tag="P", name=f"P{t}")
                    for g in groups:
                        emit_gemm_group(t, g)

                def emit_reduce(t, barrier=False):
                    # pair-add group sums -> [128, K]; AllGather (2 KB -> 16 KB)
                    s4 = agq.tile([128, K], F32, tag="s4", name=f"s4_{t}")
                    nc.gpsimd.tensor_add(s4[:], schunk[:, t, 0, :],
                                         schunk[:, t, 1, :])
                    bi = ccdr.tile([128, K], F32, tag="bi", name=f"bi{t}")
                    bo = ccdr.tile([NC, 128, K], F32, tag="bo", name=f"bo{t}")
                    nc.gpsimd.dma_start(bi[:], s4[:])
                    if barrier:
                        # re-aligning barrier: rank skew accumulates over
                        # the DMA-heavy block-0 phase (per-core HBM
                        # contention differs by ~25us). Reading bi makes
                        # each rank enter at ITS block-0 completion, so
                        # every later AllGather starts aligned and runs at
                        # its ~10us service latency instead of paying the
                        # skew as entry wait. Compute engines never block.
                        wbo2 = ccdr.tile([NC, 128, K], F32, tag="warmo",
                                         name=f"wbo2_{t}", bufs=1)
                        nc.gpsimd.collective_compute(
                            "AllGather", mybir.AluOpType.bypass,
                            replica_groups=[list(range(NC))],
                            ins=[bi[:]], outs=[wbo2[:]],
                        )
                    nc.gpsimd.collective_compute(
                        "AllGather", mybir.AluOpType.bypass,
                        replica_groups=[list(range(NC))],
                        ins=[bi[:]], outs=[bo[:]],
                    )
                    return bo

                def emit_mix(t, bo, assist=False, lngate=None):
                    # z/o DMA triggers live on the sync queue (idle in the
                    # main loop) so their waits never head-of-line block
                    # the gpsimd (CC) or scalar (Exp/Ln) queues.
                    z8 = mixp.tile([128, NC, K], F32, tag="z8", name=f"z8_{t}")
                    nc.sync.dma_start(z8[:], bo[:].rearrange("r p k -> p r k"))
                    # local pair-sum of the 8 gathered rank slices
                    nc.vector.tensor_add(z8[:, 0:4, :], z8[:, 0:4, :], z8[:, 4:8, :])
                    nc.vector.tensor_add(z8[:, 0:2, :], z8[:, 0:2, :], z8[:, 2:4, :])
                    z4 = mixp.tile([128, K], F32, tag="z4", name=f"z4_{t}")
                    nc.vector.tensor_add(z4[:], z8[:, 0, :], z8[:, 1, :])
                    a4 = mixp.tile([128, K], F32, tag="a4", name=f"a4_{t}")
                    nc.vector.reciprocal(a4[:], z4[:])
                    nc.vector.tensor_mul(a4[:], a4[:], gw[:, t, :])
                    pt = pts.pop(t)
                    red = mixp.tile([128, VSH], BF16, tag="red",
                                    name=f"red{t}", bufs=1)
                    for (c0, cw) in OCH:
                        rc = red[:, c0 : c0 + cw]
                        pk = [pt[:, k * VSH + c0 : k * VSH + c0 + cw]
                              for k in range(K)]
                        mk = mixp.tile([128, OCW], BF16, tag="mk",
                                       name=f"mk{t}_{c0}", bufs=1)
                        if assist:
                            # exposed-tail block: scalar does two of the
                            # four products so vector and scalar split the
                            # serial mix chain roughly in half
                            mks = mixp.tile([128, OCW], BF16, tag="mks",
                                            name=f"mks{t}_{c0}", bufs=1)
                            nc.scalar.mul(mks[:, :cw], pk[1], a4[:, 1:2])
                            nc.vector.tensor_scalar_mul(rc, pk[0], a4[:, 0:1])
                            nc.vector.tensor_scalar_mul(mk[:, :cw], pk[2],
                                                        a4[:, 2:3])
                            nc.vector.tensor_add(rc, rc, mk[:, :cw])
                            nc.vector.tensor_add(rc, rc, mks[:, :cw])
                            nc.scalar.mul(mk[:, :cw], pk[3], a4[:, 3:4])
                            nc.vector.tensor_add(rc, rc, mk[:, :cw])
                        else:
                            # steady state: vector-only; at distance-2 the
                            # mix has a two-block budget so its ~21us
                            # serial latency never gates the GEMM
                            for k in range(K):
                                if k == 0:
                                    nc.vector.tensor_scalar_mul(rc, pk[0],
                                                                a4[:, 0:1])
                                else:
                                    nc.vector.tensor_scalar_mul(
                                        mk[:, :cw], pk[k], a4[:, k : k + 1])
                                    nc.vector.tensor_add(rc, rc, mk[:, :cw])
                    if lngate is not None:
                        # pure dependency injection: an eps tile derived
                        # (x0 + EPS) from a late Exp of the CURRENT block,
                        # so the scheduler places the Lns late in the block
                        # where scalar has idle time, instead of between
                        # the block's early Exps (which stalls the GEMM
                        # through psum backpressure).
                        eps4 = mixp.tile([128, 1], F32, tag="eps4",
                                         name=f"eps4_{t}", bufs=1)
                        nc.vector.tensor_scalar(eps4[:], lngate, 0.0, EPS_LOG,
                                                op0=mybir.AluOpType.mult,
                                                op1=mybir.AluOpType.add)
                        lbias = eps4[:]
                    else:
                        lbias = eps_log[:]
                    for (c0, cw) in LNCH:
                        ot = mixp.tile([128, 1000], BF16, tag="ot",
                                       name=f"ot{t}_{c0}", bufs=2)
                        nc.scalar.activation(ot[:, :cw], red[:, c0 : c0 + cw],
                                             AFT.Ln, bias=lbias, scale=1.0)
                        nc.sync.dma_start(o_d[t, :, c0 : c0 + cw], ot[:, :cw])

                # ---- block 0 GEMM paced by w arrival; gate fills stalls ----
                # the gate's psum tiles come from the SAME pool/tag as the
                # GEMM psum (using a slice of a [128,2048] ring slot), so
                # they interleave with block-0's groups without exceeding
                # the 8-bank PSUM budget.
                def emit_gate_down(d):
                    slot = mmps.tile([128, 2048], F32, tag="mm",
                                     name=f"pg{d}")
                    pg = slot[:, :T]
                    for hs in range(HB // 2):
                        for half in range(2):
                            nc.tensor.matmul(
                                pg[:, half * 512 : (half + 1) * 512],
                                lhsT=wd_sb[:, 2 * hs : 2 * hs + 2,
                                           d * 128 : (d + 1) * 128],
                                rhs=xT8[:, 2 * hs : 2 * hs + 2,
                                        half * 512 : (half + 1) * 512],
                                start=(hs == 0), stop=(hs == HB // 2 - 1),
                                perf_mode=mybir.MatmulPerfMode.DoubleRow,
                            )
                    nc.vector.tensor_scalar(gT[:, d, :], pg, 1.0 / WSCALE,
                                            bd_sb[:, d, :],
                                            op0=mybir.AluOpType.mult,
                                            op1=mybir.AluOpType.add)
                    # sigmoid(y) = 1/(1+exp(-y)) via the Exp activation:
                    # keeps the gate inside the combined exp+ln table set
                    # (a Sigmoid would force ~9 act-table reloads
                    # interleaved with block-0's Exps)
                    sig = gsb.tile([128, T], F32, tag="sig", name=f"sig{d}")
                    nc.scalar.activation(sig[:], pg, AFT.Exp,
                                         bias=bdn_sb[:, d, :],
                                         scale=-1.0 / WSCALE)
                    nc.vector.tensor_scalar(sig[:], sig[:], 1.0, 1.0,
                                            op0=mybir.AluOpType.mult,
                                            op1=mybir.AluOpType.add)
                    nc.vector.reciprocal(sig[:], sig[:])
                    nc.vector.tensor_mul(gT[:, d, :], gT[:, d, :], sig[:])

                gT = gsb.tile([128, D // 128, T], BF16)
                emit_gemm(0, range(0, 2))
                for d in range(D // 128):
                    emit_gate_down(d)
                    emit_gemm(0, [2 + d])
                emit_gemm(0, range(6, NG))
                bos = {0: emit_reduce(0)}

                # gate stage B: up-proj + softmax -> gw
                gl_sb = gsb.tile([K, T], F32)
                for half in range(2):
                    slot = mmps.tile([128, 2048], F32, tag="mm",
                                     name=f"pl{half}")
                    pl = slot[:K, :512]
                    for d in range(D // 128):
                        nc.tensor.matmul(
                            pl,
                            lhsT=wu_sb[:, d, :],
                            rhs=gT[:, d, half * 512 : (half + 1) * 512],
                            start=(d == 0), stop=(d == D // 128 - 1),
                        )
                    nc.scalar.activation(gl_sb[:, half * 512 : (half + 1) * 512],
                                         pl, AFT.Identity,
                                         bias=bu_sb[:], scale=1.0)
                glt = gsb.tile([128, TB, K], F32)
                for t in range(TB):
                    slot = mmps.tile([128, 2048], F32, tag="mm",
                                     name=f"gp{t}")
                    gp = slot[:, :K]
                    nc.tensor.transpose(gp, gl_sb[:, t * 128 : (t + 1) * 128],
                                        ident32[:])
                    nc.vector.tensor_copy(glt[:, t, :], gp)
                negm = gsb.tile([128, TB], F32)
                esum = gsb.tile([128, TB], F32)
                for t in range(TB):
                    nc.vector.tensor_reduce(
                        negm[:, t : t + 1], glt[:, t, :],
                        axis=mybir.AxisListType.X, op=mybir.AluOpType.max,
                        negate=True,
                    )
                    nc.scalar.activation(gw[:, t, :], glt[:, t, :], AFT.Exp,
                                         bias=negm[:, t : t + 1], scale=1.0,
                                         accum_out=esum[:, t : t + 1])
                rsum = gsb.tile([128, TB], F32)
                nc.vector.reciprocal(rsum[:], esum[:])
                for t in range(TB):
                    nc.vector.tensor_scalar_mul(gw[:, t, :], gw[:, t, :],
                                                rsum[:, t : t + 1])
                gsb_ctx.__exit__(None, None, None)
                mixp_ctx = tc.tile_pool(name="mix", bufs=2)
                mixp = mixp_ctx.__enter__()

                # ---- main loop: GEMM + AG per block, mix two blocks
                # behind: the AllGather latency (12-20us with skew/jitter)
                # plus the ~21us mix chain get a two-block (~67us) budget,
                # so neither ever backpressures the GEMM pipeline.
                for t in range(1, TB):
                    emit_gemm(t)
                    if t > 1:
                        emit_mix(t - 2, bos.pop(t - 2),
                                 lngate=schunk[:, t, 1, 2:3])
                    bos[t] = emit_reduce(t)
                emit_mix(TB - 2, bos.pop(TB - 2), assist=True)
                emit_mix(TB - 1, bos.pop(TB - 1), assist=True)
                mixp_ctx.__exit__(None, None, None)
    nc.compile()
    return nc


_CACHE = {}


def _get_kernels():
    if "f" not in _CACHE:
        _CACHE["f"] = build_fused()
    return _CACHE["f"]


def kernel(hidden_states, rms_scale, gate_down_w, gate_down_b, gate_up_w,
           gate_up_b, expert_w, trace=False):
    nc_f = _get_kernels()
    core_ids = list(range(NC))

    x = np.asarray(hidden_states, dtype=np.float32).reshape(T, H)
    # fold the RMSNorm into host prep (rms_scale folds into the weights)
    rms = np.sqrt(np.mean(np.square(x), axis=-1, keepdims=True) + EPS_RMS)
    xh = x / rms
    # pack x-hat^T [p][hb][t]: partition p holds rows {hb*128+p}, hb-major
    xpk = np.ascontiguousarray(
        xh.T.reshape(HB, 128, T).transpose(1, 0, 2).reshape(128, HB * T)
    ).astype(FP8NP)
    scale = np.asarray(rms_scale, dtype=np.float32)
    # fold rms_scale into every weight that consumes the normed activations
    wd = (np.asarray(gate_down_w, dtype=np.float32) * scale[:, None]
          * WSCALE).astype(FP8NP)
    wu = np.asarray(gate_up_w, dtype=np.float32).astype(ml_dtypes.bfloat16)
    bd = np.ascontiguousarray(np.asarray(gate_down_b, dtype=np.float32).reshape(D, 1))
    bdn = np.ascontiguousarray(-bd)
    bu = np.ascontiguousarray(np.asarray(gate_up_b, dtype=np.float32).reshape(K, 1))
    we = np.asarray(expert_w, dtype=np.float32) * scale[:, None]
    we8 = (we * WSCALE).astype(FP8NP).reshape(HB, 128, K, V)

    in_maps = []
    for c in range(NC):
        # per column group g: [128, HB*cw] block, partition-major then
        # hb-major then columns (matches the SBUF tile layout exactly)
        blocks = []
        for (c0, cw) in GRPS:
            k, j0 = c0 // VSH, c0 % VSH
            blk = we8[:, :, k, c * VSH + j0 : c * VSH + j0 + cw]
            blocks.append(blk.transpose(1, 0, 2).reshape(128, HB * cw))
        wsh = np.ascontiguousarray(np.concatenate(blocks, axis=1))
        in_maps.append({"x": xpk, "w": wsh, "wd": wd, "wu": wu, "bd": bd,
                        "bdn": bdn, "bu": bu})

    res = run_bass_kernel_spmd(nc_f, in_maps, core_ids, trace=trace)

    out = np.empty((T, V), dtype=np.float32)
    for c in range(NC):
        out[:, c * VSH : (c + 1) * VSH] = (
            res.results[c]["o"].reshape(T, VSH).astype(np.float32))
    out = out.reshape(B, S, V)
    if trace:
        return out, (res, res)
    return out


# revision 36
# speedup vs baseline: 1.0783x; 1.0783x over previous
"""Trainium2 Bass kernel for MixtureOfSoftmaxes.

Module: RMSNorm -> gate MLP (silu, softmax over K experts) -> big GEMM
x @ expert_w (H=1024 -> K*V=128000), softmax over V per expert, mix with
gate weights, log.

Sharding: tensor-parallel over vocab. Core c owns, for all K=4 experts,
the vocab window [c*4000, (c+1)*4000). The only cross-core quantity is
the per-(token, expert) softmax denominator Z.

v3 design (single fused NEFF):
- RMSNorm folded into host prep: x-hat = x/rms(x) is computed in numpy,
  shipped pre-transposed (h on partitions) in fp8 (1 MB). No on-device
  norm, no PE transposes, no psum transpose bank.
- Block-0 GEMM starts as soon as w group 0 lands (~8us) and is paced by
  the weight-shard DMA (16 MB fp8, ~47us wire). The gate MLP's matmuls
  are emitted between block-0 groups so they fill the w-arrival stalls.
- GEMM per (token block, w group): 2 psum sub-tiles of [128,1024] f32
  (1024+1024 or 1024+928 cols), 8 DoubleRow fp8 matmuls each, then one
  Exp activation per sub-tile (psum -> fp8 P in SBUF) with accum_out
  giving quarter row-sums.
- Cross-core reduction via AllGather (4.6us floor @8 cores vs 9.7us for
  AllReduce, and single-phase so the CC queue drains ~2x faster) of the
  [128,K] partial sums; the 8 gathered slices are pair-summed locally on
  vector (3 adds). The old AllReduce measured 35-67us bi->z4 latency and
  its CC-queue serialization both stalled the GEMM pipeline mid-kernel
  and dominated the tail.
- P is kept in fp8 (16 KB/partition/block, 3 bufs); mix runs two blocks
  behind the GEMM; the two exposed tail mixes split their products
  between vector and scalar.
- Output is written bf16 (halves out-DMA bytes; ~0.2% extra rel err vs
  the 2e-2 gate) and upcast to f32 on host.
- DMA triggers that can wait (z return, outputs) live on the sync queue;
  gate-weight loads trigger from the gpsimd queue so the sync queue's
  head is free for x/w.
"""

import sys

sys.path.insert(0, "/opt/trn_rl_repo")

import numpy as np
import ml_dtypes

import concourse.bass as bass
import concourse.bacc as bacc
import concourse.mybir as mybir
import concourse.tile as tile
import concourse.hw_specs as hw_specs
import concourse.bass_interp as bass_interp
from concourse.bass_utils import run_bass_kernel_spmd
from concourse.masks import make_identity

# Steer the act-table-set selector to the combined exp+ln set: with the
# separate "exp_and_others"/"natural_log" sets visible, every Exp<->Ln
# transition in the steady loop re-loads the table (~2.7us of
# ACT_TABLE_LOAD + drain, twice per block, and the contiguous scalar
# pause stalls the GEMM through psum backpressure). Dropping those sets
# makes both functions resolve to "natural_log_exp_and_others", so the
# table is loaded once and never switched mid-loop.
import functools


@functools.cache
def _gat_combined(module_arch):
    # keep every set (ids are positional) but claim exp/ln only in the
    # combined set so the selector can't pick the single-function ones
    exp_aft = mybir.ActivationFunctionType.Exp
    ln_aft = mybir.ActivationFunctionType.Ln
    t = {}
    for name, fns in _gat_orig(module_arch).items():
        if name != "natural_log_exp_and_others":
            fns = fns - {exp_aft, ln_aft}
        t[name] = fns
    return t


_gat_orig = hw_specs.get_activation_tables
if hw_specs.get_activation_tables is not _gat_combined:
    hw_specs.get_activation_tables = _gat_combined
    bacc.get_activation_tables = _gat_combined
    bass_interp.get_activation_tables = _gat_combined

AFT = mybir.ActivationFunctionType
F32 = mybir.dt.float32
BF16 = mybir.dt.bfloat16
FP8 = mybir.dt.float8e4
FP8NP = ml_dtypes.float8_e4m3
WSCALE = 16.0

B, S, H, K, V = 2, 512, 1024, 4, 32000
T = B * S              # 1024 tokens
NC = 8                 # cores
VSH = V // NC          # 4000 vocab cols per core per expert
C = K * VSH            # 16000 GEMM cols per core (no padding)
D = H // 2             # 512 gate hidden
EPS_RMS = 1e-5
EPS_LOG = 1e-10
TB = T // 128          # 8 token blocks
HB = H // 128          # 8 contraction blocks
# w-layout column groups: per expert [0:2048] and [2048:4000]
GRPS = []
for k in range(K):
    GRPS.append((k * VSH, 2048))
    GRPS.append((k * VSH + 2048, VSH - 2048))
NG = len(GRPS)         # 8 groups
# mix sub-chunks and Ln/out chunks per vocab window
OCH = [(0, 1000), (1000, 1000), (2000, 1000), (3000, 1000)]
OCW = 1024
LNCH = [(0, 1000), (1000, 1000), (2000, 1000), (3000, 1000)]


def build_fused():
    nc = bacc.Bacc("TRN2", target_bir_lowering=False, debug=False, num_devices=NC)
    # x ships pre-normed and pre-transposed (h on partitions) in fp8:
    # partition p holds rows {hb*128+p} of x-hat^T, hb-major. One 1 MB DMA.
    x_d = nc.dram_tensor("x", [128, HB * T], FP8, kind="ExternalInput")
    # w is host-packed per-partition-contiguous: for each column group g,
    # a [128, HB*cw] block where partition p holds rows {hb*128+p} of the
    # group's columns, hb-major.
    w_d = nc.dram_tensor("w", [128, HB * C], FP8, kind="ExternalInput")
    wd_d = nc.dram_tensor("wd", [H, D], FP8, kind="ExternalInput")
    wu_d = nc.dram_tensor("wu", [D, K], BF16, kind="ExternalInput")
    bd_d = nc.dram_tensor("bd", [D, 1], F32, kind="ExternalInput")
    bdn_d = nc.dram_tensor("bdn", [D, 1], F32, kind="ExternalInput")
    bu_d = nc.dram_tensor("bu", [K, 1], F32, kind="ExternalInput")
    o_d = nc.dram_tensor("o", [TB, 128, VSH], BF16, kind="ExternalOutput")

    wd_ap = wd_d.rearrange("(hb p) d -> p hb d", p=128)
    wu_ap = wu_d.rearrange("(db p) k -> p db k", p=128)
    bd_ap = bd_d.rearrange("(db p) o -> p db o", p=128)

    with tile.TileContext(nc) as tc:
        with tc.tile_pool(name="persist", bufs=1) as pers:
            # create ALL persistent tiles up front, BEFORE any scoped pool
            # opens (later pers tiles would inherit false WAR deps from
            # scoped pools occupying the same addresses).
            w_sb = []
            for g, (c0, cw) in enumerate(GRPS):
                w_sb.append(pers.tile([128, HB, cw], FP8, name=f"wg{g}"))
            xT8 = pers.tile([128, HB, T], FP8)     # 8 KB/partition
            ident32 = pers.tile([4, 4], F32)
            make_identity(nc, ident32[:])
            eps_log = pers.tile([128, 1], F32)
            nc.gpsimd.memset(eps_log[:], EPS_LOG)
            gw = pers.tile([128, TB, K], F32)
            # group sums laid out [t, half, expert] so the per-expert
            # pair-add is a plain elementwise add of the two halves
            schunk = pers.tile([128, TB, 2, K], F32)

            with tc.tile_pool(name="pfull", bufs=3) as ppool, \
                 tc.tile_pool(name="agq", bufs=2) as agq, \
                 tc.tile_pool(name="ccdr", bufs=2, space="DRAM") as ccdr, \
                 tc.tile_pool(name="mm_psum", bufs=2, space="PSUM") as mmps:

                # gate-only SBUF lives in its own scope that closes after
                # the gate finishes; the mix pool opens after and reuses
                # its addresses (they never coexist), which is what funds
                # the third P buffer for the distance-2 pipeline.
                gsb_ctx = tc.tile_pool(name="gate_sb", bufs=1)
                gsb = gsb_ctx.__enter__()

                # warm up the collective path: the first collective pays
                # ~45us of NEFF staging + entry-barrier latency; a dummy
                # AllGather at t=0 absorbs that under the weight DMA so
                # the per-block gathers run at the ~12us steady latency.
                wbi = ccdr.tile([128, K], F32, tag="warm", name="wbi", bufs=1)
                wbo = ccdr.tile([NC, 128, K], F32, tag="warmo", name="wbo",
                                bufs=1)
                nc.gpsimd.collective_compute(
                    "AllGather", mybir.AluOpType.bypass,
                    replica_groups=[list(range(NC))],
                    ins=[wbi[:]], outs=[wbo[:]],
                )

                # ---- input DMAs ----
                # sync queue: x first (gates block-0 GEMM), then the w
                # groups. Cap descriptors at 4 KB so they round-robin
                # across all 16 DMA engines.
                nc.sync.dma_start(xT8[:], x_d[:].rearrange("p (h t) -> p h t", h=HB))
                # chain the w-group loads at depth 2: group g's trigger
                # sits behind a tiny read of group g-2 on the in-order
                # sync queue, so at most two group transfers are in
                # flight. Each group then gets ~the full aggregate DMA
                # bandwidth and completes in issue order — staggered
                # arrivals that block-0's GEMM can pipeline behind.
                # (Unchained, all 16 transfers round-robin and EVERY
                # group lands near the 47us total-wire mark.)
                off = 0
                for g, (c0, cw) in enumerate(GRPS):
                    half = HB // 2 * cw
                    if g >= 2:
                        tick = agq.tile([1, HB], FP8, tag="wtick",
                                        name=f"wtick{g}")
                        nc.sync.dma_start(tick[:], w_sb[g - 2][0:1, :, 0:1])
                    for j in range(2):
                        nc.sync.dma_start(
                            w_sb[g][:, j * HB // 2 : (j + 1) * HB // 2, :],
                            w_d[:, off + j * half : off + (j + 1) * half].rearrange(
                                "p (h c) -> p h c", h=HB // 2),
                            max_dma_last_dim=4096)
                    off += HB * cw
                # gate weights trigger from the gpsimd queue (idle early)
                wd_sb = gsb.tile([128, HB, D], FP8)   # 4 KB/partition
                nc.gpsimd.dma_start(wd_sb[:], wd_ap)
                wu_sb = gsb.tile([128, D // 128, K], BF16)
                nc.gpsimd.dma_start(wu_sb[:], wu_ap)
                bd_sb = gsb.tile([128, D // 128, 1], F32)
                nc.gpsimd.dma_start(bd_sb[:], bd_ap)
                bdn_sb = gsb.tile([128, D // 128, 1], F32)
                nc.gpsimd.dma_start(bdn_sb[:],
                                    bdn_d.rearrange("(db p) o -> p db o", p=128))
                bu_sb = gsb.tile([K, 1], F32)
                nc.gpsimd.dma_start(bu_sb[:], bu_d[:])

                pts = {}

                def emit_gemm_group(t, g):
                    pt = pts[t]
                    c0, cw = GRPS[g]
                    PT = mmps.tile([128, 2048], F32, tag="mm",
                                   name=f"mm_{t}_{g}")
                    for hs in range(HB // 2):
                        for ch0 in range(0, cw, 512):
                            chw = min(512, cw - ch0)
                            nc.tensor.matmul(
                                PT[:, ch0 : ch0 + chw],
                                lhsT=xT8[:, 2 * hs : 2 * hs + 2,
                                         t * 128 : (t + 1) * 128],
                                rhs=w_sb[g][:, 2 * hs : 2 * hs + 2,
                                            ch0 : ch0 + chw],
                                start=(hs == 0), stop=(hs == HB // 2 - 1),
                                perf_mode=mybir.MatmulPerfMode.DoubleRow,
                            )
                    nc.scalar.activation(
                        pt[:, c0 : c0 + cw], PT[:, :cw],
                        AFT.Exp, bias=0.0, scale=1.0 / WSCALE,
                        accum_out=schunk[:, t, g % 2, g // 2 : g // 2 + 1])

                def emit_gemm(t, groups=range(NG)):
                    if t not in pts:
                        pts[t] = ppool.tile([128, C], FP8, tag="P", name=f"P{t}")
                    for g in groups:
                        emit_gemm_group(t, g)

                def emit_reduce(t, barrier=False):
                    # pair-add group sums -> [128, K]; AllGather (2 KB -> 16 KB)
                    s4 = agq.tile([128, K], F32, tag="s4", name=f"s4_{t}")
                    nc.gpsimd.tensor_add(s4[:], schunk[:, t, 0, :],
                                         schunk[:, t, 1, :])
                    bi = ccdr.tile([128, K], F32, tag="bi", name=f"bi{t}")
                    bo = ccdr.tile([NC, 128, K], F32, tag="bo", name=f"bo{t}")
                    nc.gpsimd.dma_start(bi[:], s4[:])
                    if barrier:
                        # re-aligning barrier: rank skew accumulates over
                        # the DMA-heavy block-0 phase (per-core HBM
                        # contention differs by ~25us). Reading bi makes
                        # each rank enter at ITS block-0 completion, so
                        # every later AllGather starts aligned and runs at
                        # its ~10us service latency instead of paying the
                        # skew as entry wait. Compute engines never block.
                        wbo2 = ccdr.tile([NC, 128, K], F32, tag="warmo",
                                         name=f"wbo2_{t}", bufs=1)
                        nc.gpsimd.collective_compute(
                            "AllGather", mybir.AluOpType.bypass,
                            replica_groups=[list(range(NC))],
                            ins=[bi[:]], outs=[wbo2[:]],
                        )
                    nc.gpsimd.collective_compute(
                        "AllGather", mybir.AluOpType.bypass,
                        replica_groups=[list(range(NC))],
                        ins=[bi[:]], outs=[bo[:]],
                    )
                    return bo

                def emit_mix(t, bo, assist=False, lngate=None):
                    # z/o DMA triggers live on the sync queue (idle in the
                    # main loop) so their waits never head-of-line block
                    # the gpsimd (CC) or scalar (Exp/Ln) queues.
                    z8 = mixp.tile([128, NC, K], F32, tag="z8", name=f"z8_{t}")
                    nc.sync.dma_start(z8[:], bo[:].rearrange("r p k -> p r k"))
                    # local pair-sum of the 8 gathered rank slices
                    nc.vector.tensor_add(z8[:, 0:4, :], z8[:, 0:4, :], z8[:, 4:8, :])
                    nc.vector.tensor_add(z8[:, 0:2, :], z8[:, 0:2, :], z8[:, 2:4, :])
                    z4 = mixp.tile([128, K], F32, tag="z4", name=f"z4_{t}")
                    nc.vector.tensor_add(z4[:], z8[:, 0, :], z8[:, 1, :])
                    a4 = mixp.tile([128, K], F32, tag="a4", name=f"a4_{t}")
                    nc.vector.reciprocal(a4[:], z4[:])
                    nc.vector.tensor_mul(a4[:], a4[:], gw[:, t, :])
                    pt = pts.pop(t)
                    red = mixp.tile([128, VSH], BF16, tag="red",
                                    name=f"red{t}", bufs=1)
                    for (c0, cw) in OCH:
                        rc = red[:, c0 : c0 + cw]
                        pk = [pt[:, k * VSH + c0 : k * VSH + c0 + cw]
                              for k in range(K)]
                        mk = mixp.tile([128, OCW], BF16, tag="mk",
                                       name=f"mk{t}_{c0}", bufs=1)
                        if assist:
                            # exposed-tail block: scalar does two of the
                            # four products so vector and scalar split the
                            # serial mix chain roughly in half
                            mks = mixp.tile([128, OCW], BF16, tag="mks",
                                            name=f"mks{t}_{c0}", bufs=1)
                            nc.scalar.mul(mks[:, :cw], pk[1], a4[:, 1:2])
                            nc.vector.tensor_scalar_mul(rc, pk[0], a4[:, 0:1])
                            nc.vector.tensor_scalar_mul(mk[:, :cw], pk[2],
                                                        a4[:, 2:3])
                            nc.vector.tensor_add(rc, rc, mk[:, :cw])
                            nc.vector.tensor_add(rc, rc, mks[:, :cw])
                            nc.scalar.mul(mk[:, :cw], pk[3], a4[:, 3:4])
                            nc.vector.tensor_add(rc, rc, mk[:, :cw])
                        else:
                            # steady state: vector-only; at distance-2 the
                            # mix has a two-block budget so its ~21us
                            # serial latency never gates the GEMM
                            for k in range(K):
                                if k == 0:
                                    nc.vector.tensor_scalar_mul(rc, pk[0],
                                                                a4[:, 0:1])
                                else:
                                    nc.vector.tensor_scalar_mul(
                                        mk[:, :cw], pk[k], a4[:, k : k + 1])
                                    nc.vector.tensor_add(rc, rc, mk[:, :cw])
                    if lngate is not None:
                        # pure dependency injection: an eps tile derived
                        # (x0 + EPS) from a late Exp of the CURRENT block,
                        # so the scheduler places the Lns late in the block
                        # where scalar has idle time, instead of between
                        # the block's early Exps (which stalls the GEMM
                        # through psum backpressure).
                        eps4 = mixp.tile([128, 1], F32, tag="eps4",
                                         name=f"eps4_{t}", bufs=1)
                        nc.vector.tensor_scalar(eps4[:], lngate, 0.0, EPS_LOG,
                                                op0=mybir.AluOpType.mult,
                                                op1=mybir.AluOpType.add)
                        lbias = eps4[:]
                    else:
                        lbias = eps_log[:]
                    for (c0, cw) in LNCH:
                        ot = mixp.tile([128, 1000], BF16, tag="ot",
                                       name=f"ot{t}_{c0}", bufs=2)
                        nc.scalar.activation(ot[:, :cw], red[:, c0 : c0 + cw],
                                             AFT.Ln, bias=lbias, scale=1.0)
                        nc.sync.dma_start(o_d[t, :, c0 : c0 + cw], ot[:, :cw])

                # ---- block 0 GEMM paced by w arrival; gate fills stalls ----
                # the gate's psum tiles come from the SAME pool/tag as the
                # GEMM psum (using a slice of a [128,2048] ring slot), so
                # they interleave with block-0's groups without exceeding
                # the 8-bank PSUM budget.
                def emit_gate_down(d):
                    slot = mmps.tile([128, 2048], F32, tag="mm",
                                     name=f"pg{d}")
                    pg = slot[:, :T]
                    for hs in range(HB // 2):
                        for half in range(2):
                            nc.tensor.matmul(
                                pg[:, half * 512 : (half + 1) * 512],
                                lhsT=wd_sb[:, 2 * hs : 2 * hs + 2,
                                           d * 128 : (d + 1) * 128],
                                rhs=xT8[:, 2 * hs : 2 * hs + 2,
                                        half * 512 : (half + 1) * 512],
                                start=(hs == 0), stop=(hs == HB // 2 - 1),
                                perf_mode=mybir.MatmulPerfMode.DoubleRow,
                            )
                    nc.vector.tensor_scalar(gT[:, d, :], pg, 1.0 / WSCALE,
                                            bd_sb[:, d, :],
                                            op0=mybir.AluOpType.mult,
                                            op1=mybir.AluOpType.add)
                    # sigmoid(y) = 1/(1+exp(-y)) via the Exp activation:
                    # keeps the gate inside the combined exp+ln table set
                    # (a Sigmoid would force ~9 act-table reloads
                    # interleaved with block-0's Exps)
                    sig = gsb.tile([128, T], F32, tag="sig", name=f"sig{d}")
                    nc.scalar.activation(sig[:], pg, AFT.Exp,
                                         bias=bdn_sb[:, d, :],
                                         scale=-1.0 / WSCALE)
                    nc.vector.tensor_scalar(sig[:], sig[:], 1.0, 1.0,
                                            op0=mybir.AluOpType.mult,
                                            op1=mybir.AluOpType.add)
                    nc.vector.reciprocal(sig[:], sig[:])
                    nc.vector.tensor_mul(gT[:, d, :], gT[:, d, :], sig[:])

                gT = gsb.tile([128, D // 128, T], BF16)
                emit_gemm(0, range(0, 2))
                for d in range(D // 128):
                    emit_gate_down(d)
                    emit_gemm(0, [2 + d])
                emit_gemm(0, range(6, NG))
                bos = {0: emit_reduce(0)}

                # gate stage B: up-proj + softmax -> gw
                gl_sb = gsb.tile([K, T], F32)
                for half in range(2):
                    slot = mmps.tile([128, 2048], F32, tag="mm",
                                     name=f"pl{half}")
                    pl = slot[:K, :512]
                    for d in range(D // 128):
                        nc.tensor.matmul(
                            pl,
                            lhsT=wu_sb[:, d, :],
                            rhs=gT[:, d, half * 512 : (half + 1) * 512],
                            start=(d == 0), stop=(d == D // 128 - 1),
                        )
                    nc.scalar.activation(gl_sb[:, half * 512 : (half + 1) * 512],
                                         pl, AFT.Identity,
                                         bias=bu_sb[:], scale=1.0)
                glt = gsb.tile([128, TB, K], F32)
                for t in range(TB):
                    slot = mmps.tile([128, 2048], F32, tag="mm",
                                     name=f"gp{t}")
                    gp = slot[:, :K]
                    nc.tensor.transpose(gp, gl_sb[:, t * 128 : (t + 1) * 128],
                                        ident32[:])
                    nc.vector.tensor_copy(glt[:, t, :], gp)
                negm = gsb.tile([128, TB], F32)
                esum = gsb.tile([128, TB], F32)
                for t in range(TB):
                    nc.vector.tensor_reduce(
                        negm[:, t : t + 1], glt[:, t, :],
                        axis=mybir.AxisListType.X, op=mybir.AluOpType.max,
                        negate=True,
                    )
                    nc.scalar.activation(gw[:, t, :], glt[:, t, :], AFT.Exp,
                                         bias=negm[:, t : t + 1], scale=1.0,
                                         accum_out=esum[:, t : t + 1])
                rsum = gsb.tile([128, TB], F32)
                nc.vector.reciprocal(rsum[:], esum[:])
                for t in range(TB):
                    nc.vector.tensor_scalar_mul(gw[:, t, :], gw[:, t, :],
                                                rsum[:, t : t + 1])
                gsb_ctx.__exit__(None, None, None)
                mixp_ctx = tc.tile_pool(name="mix", bufs=2)
                mixp = mixp_ctx.__enter__()

                # ---- main loop: GEMM + AG per block, mix two blocks
                # behind: the AllGather latency (12-20us with skew/jitter)
                # plus the ~21us mix chain get a two-block (~67us) budget,
                # so neither ever backpressures the GEMM pipeline.
                for t in range(1, TB):
                    emit_gemm(t)
                    if t > 1:
                        emit_mix(t - 2, bos.pop(t - 2),
                                 lngate=schunk[:, t, 1, 2:3])
                    bos[t] = emit_reduce(t)
                emit_mix(TB - 2, bos.pop(TB - 2), assist=True)
                emit_mix(TB - 1, bos.pop(TB - 1), assist=True)
                mixp_ctx.__exit__(None, None, None)
    nc.compile()
    return nc


_CACHE = {}


def _get_kernels():
    if "f" not in _CACHE:
        _CACHE["f"] = build_fused()
    return _CACHE["f"]


def kernel(hidden_states, rms_scale, gate_down_w, gate_down_b, gate_up_w,
           gate_up_b, expert_w, trace=False):
    nc_f = _get_kernels()
    core_ids = list(range(NC))

    x = np.asarray(hidden_states, dtype=np.float32).reshape(T, H)
    # fold the RMSNorm into host prep (rms_scale folds into the weights)
    rms = np.sqrt(np.mean(np.square(x), axis=-1, keepdims=True) + EPS_RMS)
    xh = x / rms
    # pack x-hat^T [p][hb][t]: partition p holds rows {hb*128+p}, hb-major
    xpk = np.ascontiguousarray(
        xh.T.reshape(HB, 128, T).transpose(1, 0, 2).reshape(128, HB * T)
    ).astype(FP8NP)
    scale = np.asarray(rms_scale, dtype=np.float32)
    # fold rms_scale into every weight that consumes the normed activations
    wd = (np.asarray(gate_down_w, dtype=np.float32) * scale[:, None]
          * WSCALE).astype(FP8NP)
    wu = np.asarray(gate_up_w, dtype=np.float32).astype(ml_dtypes.bfloat16)
    bd = np.ascontiguousarray(np.asarray(gate_down_b, dtype=np.float32).reshape(D, 1))
    bdn = np.ascontiguousarray(-bd)
    bu = np.ascontiguousarray(np.asarray(gate_up_b, dtype=np.float32).reshape(K, 1))
    we = np.asarray(expert_w, dtype=np.float32) * scale[:, None]
    we8 = (we * WSCALE).astype(FP8NP).reshape(HB, 128, K, V)

    in_maps = []
    for c in range(NC):
        # per column group g: [128, HB*cw] block, partition-major then
        # hb-major then columns (matches the SBUF tile layout exactly)
        blocks = []
        for (c0, cw) in GRPS:
            k, j0 = c0 // VSH, c0 % VSH
            blk = we8[:, :, k, c * VSH + j0 : c * VSH + j0 + cw]
            blocks.append(blk.transpose(1, 0, 2).reshape(128, HB * cw))
        wsh = np.ascontiguousarray(np.concatenate(blocks, axis=1))
        in_maps.append({"x": xpk, "w": wsh, "wd": wd, "wu": wu, "bd": bd,
                        "bdn": bdn, "bu": bu})

    res = run_bass_kernel_spmd(nc_f, in_maps, core_ids, trace=trace)

    out = np.empty((T, V), dtype=np.float32)
    for c in range(NC):
        out[:, c * VSH : (c + 1) * VSH] = (
            res.results[c]["o"].reshape(T, VSH).astype(np.float32))
    out = out.reshape(B, S, V)
    if trace:
        return out, (res, res)
    return out
